# revision 10
# baseline (speedup 1.0000x reference)
"""Causal self-attention on 8 TRN2 NeuronCores.

Sharding: core c -> (batch b = c//2, head-group g = c%2).
B=4, T=2048, D=1024, 16 heads x 64. Each core computes attention for its
batch and its 8 heads, plus the partial output projection for those heads;
the host sums the two partial projections per batch.

Device layouts (host pre-transposes everything, all bf16):
  xT    [1024, 2048]  x[b].T
  wqkT  [1024, 1024]  cols 0..511 q-feats, 512..1023 k-feats (group g)
  wvT   [1024, 512]   v-feats (group g)
  wpT   [512, 1024]   w_proj[:, g*512:(g+1)*512].T
  mask  [128, 2048]   4 causal patterns of [128,512] for diag offsets 0/128/256/384
Output: yT [1024, 2048] bf16 partial y[b].T (sum over this core's heads).

Attention runs in S^T orientation (k on partitions, q on free dim). The two
heads of a pair (2p, 2p+1) live on SBUF partitions 0-63 / 64-127 of the
same qk tiles, and each k-block piece computes BOTH heads' scores into one
[128,1024] PSUM tile (A in cols 0-511, B in 512-1023) so a single ACT exp
covers the pair, the s tile double-buffers in 4 PSUM banks, and the K=64
matmuls land on PE row-tiles (0,0)/(64,0). P^T V accumulates per head with
an all-ones V column producing softmax denominators in row 64.
Normalization: DVE fast-reciprocal on both heads' denominators at once ->
GpSimd partition-broadcast -> DVE multiply on evacuation.

Scheduling: query chunks 0/1 only need the first half of q/k/v, so the
second half of the QKV projection is emitted after chunk 1 and fills PE
gaps while ACT grinds exps; the output projection for a query column is
emitted right after its chunk completes for the same reason.
"""

import sys

for _p in ("/opt/pypackages", "/opt/trn_rl_repo"):
    if _p not in sys.path:
        sys.path.insert(0, _p)

from contextlib import ExitStack

import ml_dtypes
import numpy as np

import concourse.bass as bass
import concourse.tile as tile
from concourse import bacc, mybir
from concourse.bass_utils import run_bass_kernel_spmd

F32 = mybir.dt.float32
BF16 = mybir.dt.bfloat16
AF = mybir.ActivationFunctionType
OP = mybir.AluOpType

D = 1024
T = 2048
NH_LOC = 8          # heads per core
DH = 64
GF = NH_LOC * DH    # 512 features per group

LAST_RESULTS = None
_CACHED = None


def build_program():
    nc = bacc.Bacc("TRN2", target_bir_lowering=False, debug=False)

    xT_d = nc.dram_tensor("xT", [D, T], BF16, kind="ExternalInput").ap()
    wqk_d = nc.dram_tensor("wqkT", [D, 2 * GF], BF16, kind="ExternalInput").ap()
    wv_d = nc.dram_tensor("wvT", [D, GF], BF16, kind="ExternalInput").ap()
    wp_d = nc.dram_tensor("wpT", [GF, D], BF16, kind="ExternalInput").ap()
    mask_d = nc.dram_tensor("mask", [128, 2048], BF16, kind="ExternalInput").ap()
    ones_d = nc.dram_tensor("ones8", [128, 8], BF16, kind="ExternalInput").ap()
    yT_d = nc.dram_tensor("yT", [D, T], BF16, kind="ExternalOutput").ap()

    with tile.TileContext(nc) as tc:
        with ExitStack() as octx:
            # ---- persistent pools --------------------------------------
            qk_pool = octx.enter_context(tc.tile_pool(name="qkT", bufs=1))
            v_pool = octx.enter_context(tc.tile_pool(name="vN", bufs=1))
            o_pool = octx.enter_context(tc.tile_pool(name="outT", bufs=1))
            c_pool = octx.enter_context(tc.tile_pool(name="const", bufs=1))
            w_pool = octx.enter_context(tc.tile_pool(name="wqk", bufs=1))
            wv_pool = octx.enter_context(tc.tile_pool(name="wv", bufs=1))
            wp_pool = octx.enter_context(tc.tile_pool(name="wp", bufs=1))
            x_pool = octx.enter_context(tc.tile_pool(name="xT", bufs=2))
            y_pool = octx.enter_context(tc.tile_pool(name="y", bufs=4))

            # qk: per-half tiles [128,1024]; m 0..3 q-feats, m 4..7 k-feats
            qkh = [[qk_pool.tile([128, 1024], BF16, name=f"qk{h}_{m}", tag=f"qk{h}_{m}")
                    for m in range(8)] for h in range(2)]
            # vN: 16 tiles [128, 520]; cols h*65+0..63 v-feats, col h*65+64 ones
            v_t = [v_pool.tile([128, 8 * (DH + 1)], BF16, name=f"v{t}", tag=f"v{t}") for t in range(16)]
            # attention outputs per (pair, chunk): [128, 512]
            oc_t = [[o_pool.tile([128, 512], BF16, name=f"o{p}_{c}", tag=f"o{p}_{c}")
                     for c in range(4)] for p in range(4)]

            # ---- input DMAs (mask first for PE warmup, then x/wqk
            # interleaved so the first accumulation can start ASAP) -------
            mask_t = c_pool.tile([128, 2048], BF16, name="mask", tag="mask")
            nc.sync.dma_start(mask_t[:], mask_d[:])
            x_h0 = [x_pool.tile([128, 1024], BF16, name=f"x{k}", tag=f"x{k}") for k in range(8)]
            wqk_t = [w_pool.tile([128, 2 * GF], BF16, name=f"wqk{k}", tag=f"wqk{k}") for k in range(8)]
            for k in range(8):
                nc.sync.dma_start(x_h0[k][:], xT_d[k * 128:(k + 1) * 128, 0:1024])
                nc.sync.dma_start(wqk_t[k][:], wqk_d[k * 128:(k + 1) * 128, :])
            wv_t = [wv_pool.tile([128, GF], BF16, name=f"wv{k}", tag=f"wv{k}") for k in range(8)]
            for k in range(8):
                nc.sync.dma_start(wv_t[k][:], wv_d[k * 128:(k + 1) * 128, :])
            for t in range(16):
                dst = v_t[t][:].rearrange("p (h e) -> p h e", h=8, e=65)[:, :, 64:65]
                nc.sync.dma_start(dst, ones_d[:].unsqueeze(2))
            wp_t = [wp_pool.tile([128, D], BF16, name=f"wp{k}", tag=f"wp{k}") for k in range(4)]
            for k in range(4):
                nc.sync.dma_start(wp_t[k][:], wp_d[k * 128:(k + 1) * 128, :])

            def qkv_m(half, x_half, ps_qk, m, wide):
                """q/k features m for one T-half."""
                if wide:
                    ps = ps_qk.tile([128, 1024], F32, name="psqk", tag="psqk")
                    for k in range(8):
                        for n in range(2):
                            nc.tensor.matmul(
                                ps[:, n * 512:(n + 1) * 512],
                                (wqk_t[k][:, m * 128:(m + 1) * 128]),
                                (x_half[k][:, n * 512:(n + 1) * 512]),
                                start=(k == 0), stop=(k == 7),
                                skip_group_check=True,
                            )
                    nc.vector.tensor_copy(qkh[half][m][:], ps[:])
                else:
                    for n in range(2):
                        ps = ps_qk.tile([128, 512], F32, name="psqk", tag="psqk")
                        for k in range(8):
                            nc.tensor.matmul(
                                ps[:],
                                (wqk_t[k][:, m * 128:(m + 1) * 128]),
                                (x_half[k][:, n * 512:(n + 1) * 512]),
                                start=(k == 0), stop=(k == 7),
                                skip_group_check=True,
                            )
                        dst = qkh[half][m][:, n * 512:(n + 1) * 512]
                        nc.vector.tensor_copy(dst, ps[:])

            def v_tt(half, x_half, ps_v, tt, tag):
                psv = ps_v.tile([128, 512], F32, name="psv", tag=tag)
                for k in range(8):
                    nc.tensor.matmul(
                        psv[:],
                        (x_half[k][:, tt * 128:(tt + 1) * 128]),
                        (wv_t[k][:]),
                        start=(k == 0), stop=(k == 7),
                        skip_group_check=True,
                    )
                vt = v_t[half * 8 + tt]
                src = psv[:].rearrange("p (h e) -> p h e", h=8, e=64)
                dst = vt[:].rearrange("p (h e) -> p h e", h=8, e=65)[:, :, 0:64]
                nc.vector.tensor_copy(dst, src)

            def qkv_half(half, x_half, ps_qk, ps_v, n_grp):
                """Project one T-half: q/k into qkh[half], v into v_t."""
                for m in range(8):
                    qkv_m(half, x_half, ps_qk, m, wide=(n_grp == 2))
                for tt in range(8):
                    v_tt(half, x_half, ps_v, tt,
                         "psqk" if ps_v is ps_qk else "psv")

            # ================= phase 1 first half =======================
            with ExitStack() as p1:
                ps_qk = p1.enter_context(tc.tile_pool(name="ps_qk", bufs=2, space="PSUM"))
                ps_v = p1.enter_context(tc.tile_pool(name="ps_v", bufs=2, space="PSUM"))
                # HAM warmup: ~4us of matmuls on the mask tile while the
                # x/wqk DMAs stream in, so real matmuls start at 2.4 GHz
                psw = ps_qk.tile([128, 512], F32, name="psw", tag="psqk")
                for i in range(18):
                    nc.tensor.matmul(
                        psw[:], (mask_t[:, 0:128]), (mask_t[:, 512:1024]),
                        start=(i == 0), stop=(i == 17), skip_group_check=True,
                    )
                qkv_half(0, x_h0, ps_qk, ps_v, n_grp=2)

            # x second half: emit DMAs now so they run during attention
            x_h1 = [x_pool.tile([128, 1024], BF16, name=f"x{k}", tag=f"x{k}") for k in range(8)]
            for k in range(8):
                nc.sync.dma_start(x_h1[k][:], xT_d[k * 128:(k + 1) * 128, 1024:2048])

            # ====== attention + second-half projection + output proj ====
            with ExitStack() as p2:
                ps_s = p2.enter_context(tc.tile_pool(name="ps_s", bufs=2, space="PSUM"))
                ps_av = p2.enter_context(tc.tile_pool(name="ps_av", bufs=3, space="PSUM"))
                pt_pool = p2.enter_context(tc.tile_pool(name="pt", bufs=3))
                r_pool = p2.enter_context(tc.tile_pool(name="recip", bufs=2))

                def attn_chunk(c, pair):
                    """Both heads of `pair` for query chunk c."""
                    npieces = 4 * c + 4
                    hA, hB = 2 * pair, 2 * pair + 1
                    avA = ps_av.tile([65, 512], F32, name="avA", tag="av")
                    avB = ps_av.tile([65, 512], F32, name="avB", tag="av")
                    qh, qo = c // 2, (c % 2) * 512
                    for j in range(npieces):
                        kh, ko = j // 8, (j % 8) * 128
                        # diagonal blocks: columns left of the diagonal are
                        # all-masked -> only compute q >= off
                        off = (j - 4 * c) * 128 if j // 4 == c else 0
                        w = 512 - off
                        s = ps_s.tile([128, 1024], F32, name="s", tag="s")
                        nc.tensor.matmul(
                            s[:, off:512],
                            (qkh[kh][4 + pair][0:64, ko:ko + 128]),
                            (qkh[qh][pair][0:64, qo + off:qo + 512]),
                            start=True, stop=True, skip_group_check=True,
                        )
                        nc.tensor.matmul(
                            s[:, 512 + off:1024],
                            (qkh[kh][4 + pair][64:128, ko:ko + 128]),
                            (qkh[qh][pair][64:128, qo + off:qo + 512]),
                            start=True, stop=True, skip_group_check=True,
                        )
                        pt = pt_pool.tile([128, 1024], BF16, name="pt", tag="pt")
                        if off == 0:
                            nc.scalar.activation(pt[:], s[:], AF.Exp, scale=0.125)
                        else:
                            sv = s[:].rearrange("p (t q) -> p t q", t=2, q=512)
                            pv = pt[:].rearrange("p (t q) -> p t q", t=2, q=512)
                            nc.scalar.activation(
                                pv[:, :, off:512], sv[:, :, off:512],
                                AF.Exp, scale=0.125,
                            )
                        if j // 4 == c:  # diagonal block -> causal mask
                            p = j - 4 * c
                            for half in range(2):
                                nc.vector.tensor_tensor(
                                    pt[:, half * 512 + off:half * 512 + 512],
                                    pt[:, half * 512 + off:half * 512 + 512],
                                    mask_t[:, p * 512 + off:(p + 1) * 512],
                                    op=OP.mult,
                                )
                        nc.tensor.matmul(
                            avA[:, off:512],
                            (v_t[j][:, hA * 65:(hA + 1) * 65]),
                            (pt[:, off:512]),
                            start=(j == 0), stop=(j == npieces - 1),
                            skip_group_check=True,
                        )
                        nc.tensor.matmul(
                            avB[:, off:512],
                            (v_t[j][:, hB * 65:(hB + 1) * 65]),
                            (pt[:, 512 + off:1024]),
                            start=(j == 0), stop=(j == npieces - 1),
                            skip_group_check=True,
                        )
                    # normalize + evacuate both heads
                    den = r_pool.tile([1, 1024], F32, name="den", tag="den")
                    nc.vector.tensor_copy(den[0:1, 0:512], avA[64:65, :])
                    nc.vector.tensor_copy(den[0:1, 512:1024], avB[64:65, :])
                    rec = r_pool.tile([1, 1024], F32, name="rec", tag="rec")
                    nc.vector.reciprocal_approx_fast(rec[:], den[:])
                    rbA = r_pool.tile([64, 512], F32, name="rbA", tag="rbA")
                    rbB = r_pool.tile([64, 512], F32, name="rbB", tag="rbB")
                    nc.gpsimd.partition_broadcast(rbA[:], rec[0:1, 0:512])
                    nc.gpsimd.partition_broadcast(rbB[:], rec[0:1, 512:1024])
                    nc.vector.tensor_tensor(
                        oc_t[pair][c][0:64, :], avA[0:64, :], rbA[:], op=OP.mult,
                    )
                    nc.vector.tensor_tensor(
                        oc_t[pair][c][64:128, :], avB[0:64, :], rbB[:], op=OP.mult,
                    )

                def proj_col(c, ps_y):
                    """Output projection for query column c."""
                    for m in range(8):
                        psy = ps_y.tile([128, 512], F32, name="psy", tag="psy")
                        for kk in range(4):
                            nc.tensor.matmul(
                                psy[:],
                                (wp_t[kk][:, m * 128:(m + 1) * 128]),
                                (oc_t[kk][c][:]),
                                start=(kk == 0), stop=(kk == 3),
                                skip_group_check=True,
                            )
                        yt = y_pool.tile([128, 512], BF16, name="yst", tag="yst")
                        nc.vector.tensor_copy(yt[:], psy[:])
                        nc.sync.dma_start(
                            yT_d[m * 128:(m + 1) * 128, c * 512:(c + 1) * 512], yt[:]
                        )

                for pair in range(4):
                    attn_chunk(0, pair)

                # second-half projection interleaved with chunk-1 attention:
                # fills PE gaps while ACT grinds the exps
                with ExitStack() as p1b:
                    ps_qk1 = p1b.enter_context(tc.tile_pool(name="ps_qk1", bufs=1, space="PSUM"))
                    for pair in range(4):
                        attn_chunk(1, pair)
                        qkv_m(1, x_h1, ps_qk1, pair, wide=False)      # q tile for pair
                        qkv_m(1, x_h1, ps_qk1, 4 + pair, wide=False)  # k tile for pair
                        v_tt(1, x_h1, ps_qk1, 2 * pair, "psqk")
                        v_tt(1, x_h1, ps_qk1, 2 * pair + 1, "psqk")

                with ExitStack() as p3:
                    ps_y = p3.enter_context(tc.tile_pool(name="ps_y", bufs=1, space="PSUM"))
                    proj_col(0, ps_y)
                    for pair in range(4):
                        attn_chunk(2, pair)
                    proj_col(1, ps_y)
                    for pair in range(2):
                        attn_chunk(3, pair)
                    proj_col(2, ps_y)
                    for pair in range(2, 4):
                        attn_chunk(3, pair)

            # final column after the attention scope closes: PSUM is free,
            # so give the accumulators room to pipeline
            with ExitStack() as p4:
                ps_y2 = p4.enter_context(tc.tile_pool(name="ps_y2", bufs=3, space="PSUM"))
                for m in range(8):
                    psy = ps_y2.tile([128, 512], F32, name="psy", tag="psy")
                    for kk in range(4):
                        nc.tensor.matmul(
                            psy[:],
                            (wp_t[kk][:, m * 128:(m + 1) * 128]),
                            (oc_t[kk][3][:]),
                            start=(kk == 0), stop=(kk == 3),
                            skip_group_check=True,
                        )
                    yt = y_pool.tile([128, 512], BF16, name="yst", tag="yst")
                    nc.vector.tensor_copy(yt[:], psy[:])
                    nc.sync.dma_start(
                        yT_d[m * 128:(m + 1) * 128, 3 * 512:4 * 512], yt[:]
                    )

    nc.compile()
    return nc


def _make_mask():
    mask = np.zeros((128, 2048), dtype=np.float32)
    kk = np.arange(128)[:, None]
    q = np.arange(512)[None, :]
    for p in range(4):
        d = 128 * p
        mask[:, p * 512:(p + 1) * 512] = ((q - d) >= kk).astype(np.float32)
    return mask


def kernel(x, w_qkv, w_proj):
    global LAST_RESULTS, _CACHED
    x = np.asarray(x, dtype=np.float32)
    w_qkv = np.asarray(w_qkv, dtype=np.float32)
    w_proj = np.asarray(w_proj, dtype=np.float32)
    B = x.shape[0]

    if _CACHED is None:
        _CACHED = build_program()
    nc = _CACHED

    bf16 = ml_dtypes.bfloat16
    mask = _make_mask()
    in_maps = []
    for c in range(8):
        b, g = c // 2, c % 2
        wq = w_qkv[g * GF:(g + 1) * GF, :]                # [512, 1024]
        wk = w_qkv[D + g * GF: D + (g + 1) * GF, :]
        wv = w_qkv[2 * D + g * GF: 2 * D + (g + 1) * GF, :]
        in_maps.append({
            "xT": np.ascontiguousarray(x[b].T).astype(bf16),
            "wqkT": np.ascontiguousarray(np.concatenate([wq, wk], axis=0).T).astype(bf16),
            "wvT": np.ascontiguousarray(wv.T).astype(bf16),
            "wpT": np.ascontiguousarray(w_proj[:, g * GF:(g + 1) * GF].T).astype(bf16),
            "mask": mask.astype(bf16),
            "ones8": np.ones((128, 8), bf16),
        })

    res = run_bass_kernel_spmd(nc, in_maps, core_ids=list(range(8)))
    LAST_RESULTS = res

    y = np.empty_like(x)
    for b in range(B):
        yT = (res.results[2 * b]["yT"].astype(np.float32)
              + res.results[2 * b + 1]["yT"].astype(np.float32))
        y[b] = yT.T
    return y


# revision 11
# speedup vs baseline: 1.0046x; 1.0046x over previous
"""Causal self-attention on 8 TRN2 NeuronCores.

Sharding: core c -> (batch b = c//2, head-group g = c%2).
B=4, T=2048, D=1024, 16 heads x 64. Each core computes attention for its
batch and its 8 heads, plus the partial output projection for those heads;
the host sums the two partial projections per batch.

Device layouts (host pre-transposes everything, all bf16):
  xT    [1024, 2048]  x[b].T
  wqkT  [1024, 1024]  cols 0..511 q-feats, 512..1023 k-feats (group g)
  wvT   [1024, 512]   v-feats (group g)
  wpT   [512, 1024]   w_proj[:, g*512:(g+1)*512].T
  mask  [128, 2048]   4 causal patterns of [128,512] for diag offsets 0/128/256/384
Output: yT [1024, 2048] bf16 partial y[b].T (sum over this core's heads).

Attention runs in S^T orientation (k on partitions, q on free dim). The two
heads of a pair (2p, 2p+1) live on SBUF partitions 0-63 / 64-127 of the
same qk tiles, and each k-block piece computes BOTH heads' scores into one
[128,1024] PSUM tile (A in cols 0-511, B in 512-1023) so a single ACT exp
covers the pair, the s tile double-buffers in 4 PSUM banks, and the K=64
matmuls land on PE row-tiles (0,0)/(64,0). P^T V accumulates per head with
an all-ones V column producing softmax denominators in row 64.
Normalization: DVE fast-reciprocal on both heads' denominators at once ->
GpSimd partition-broadcast -> DVE multiply on evacuation.

Scheduling: query chunks 0/1 only need the first half of q/k/v, so the
second half of the QKV projection is emitted after chunk 1 and fills PE
gaps while ACT grinds exps; the output projection for a query column is
emitted right after its chunk completes for the same reason.
"""

import sys

for _p in ("/opt/pypackages", "/opt/trn_rl_repo"):
    if _p not in sys.path:
        sys.path.insert(0, _p)

from contextlib import ExitStack

import ml_dtypes
import numpy as np

import concourse.bass as bass
import concourse.tile as tile
from concourse import bacc, mybir
from concourse.bass_utils import run_bass_kernel_spmd

F32 = mybir.dt.float32
BF16 = mybir.dt.bfloat16
AF = mybir.ActivationFunctionType
OP = mybir.AluOpType

D = 1024
T = 2048
NH_LOC = 8          # heads per core
DH = 64
GF = NH_LOC * DH    # 512 features per group

LAST_RESULTS = None
_CACHED = None


def build_program():
    nc = bacc.Bacc("TRN2", target_bir_lowering=False, debug=False)

    xT_d = nc.dram_tensor("xT", [D, T], BF16, kind="ExternalInput").ap()
    wqk_d = nc.dram_tensor("wqkT", [D, 2 * GF], BF16, kind="ExternalInput").ap()
    wv_d = nc.dram_tensor("wvT", [D, GF], BF16, kind="ExternalInput").ap()
    wp_d = nc.dram_tensor("wpT", [GF, D], BF16, kind="ExternalInput").ap()
    mask_d = nc.dram_tensor("mask", [128, 2048], BF16, kind="ExternalInput").ap()
    ones_d = nc.dram_tensor("ones8", [128, 8], BF16, kind="ExternalInput").ap()
    yT_d = nc.dram_tensor("yT", [D, T], BF16, kind="ExternalOutput").ap()

    with tile.TileContext(nc) as tc:
        with ExitStack() as octx:
            # ---- persistent pools --------------------------------------
            qk_pool = octx.enter_context(tc.tile_pool(name="qkT", bufs=1))
            v_pool = octx.enter_context(tc.tile_pool(name="vN", bufs=1))
            o_pool = octx.enter_context(tc.tile_pool(name="outT", bufs=1))
            c_pool = octx.enter_context(tc.tile_pool(name="const", bufs=1))
            w_pool = octx.enter_context(tc.tile_pool(name="wqk", bufs=1))
            wv_pool = octx.enter_context(tc.tile_pool(name="wv", bufs=1))
            wp_pool = octx.enter_context(tc.tile_pool(name="wp", bufs=1))
            x_pool = octx.enter_context(tc.tile_pool(name="xT", bufs=2))
            y_pool = octx.enter_context(tc.tile_pool(name="y", bufs=4))

            # qk: per-half tiles [128,1024]; m 0..3 q-feats, m 4..7 k-feats
            qkh = [[qk_pool.tile([128, 1024], BF16, name=f"qk{h}_{m}", tag=f"qk{h}_{m}")
                    for m in range(8)] for h in range(2)]
            # vN: 16 tiles [128, 520]; cols h*65+0..63 v-feats, col h*65+64 ones
            v_t = [v_pool.tile([128, 8 * (DH + 1)], BF16, name=f"v{t}", tag=f"v{t}") for t in range(16)]
            # attention outputs per (pair, chunk): [128, 512]
            oc_t = [[o_pool.tile([128, 512], BF16, name=f"o{p}_{c}", tag=f"o{p}_{c}")
                     for c in range(4)] for p in range(4)]

            # ---- input DMAs (mask first for PE warmup, then x/wqk
            # interleaved so the first accumulation can start ASAP) -------
            mask_t = c_pool.tile([128, 2048], BF16, name="mask", tag="mask")
            nc.sync.dma_start(mask_t[:], mask_d[:])
            x_h0 = [x_pool.tile([128, 1024], BF16, name=f"x{k}", tag=f"x{k}") for k in range(8)]
            wqk_t = [w_pool.tile([128, 2 * GF], BF16, name=f"wqk{k}", tag=f"wqk{k}") for k in range(8)]
            for k in range(8):
                nc.sync.dma_start(x_h0[k][:], xT_d[k * 128:(k + 1) * 128, 0:1024])
                nc.sync.dma_start(wqk_t[k][:], wqk_d[k * 128:(k + 1) * 128, :])
            wv_t = [wv_pool.tile([128, GF], BF16, name=f"wv{k}", tag=f"wv{k}") for k in range(8)]
            for k in range(8):
                nc.sync.dma_start(wv_t[k][:], wv_d[k * 128:(k + 1) * 128, :])
            for t in range(16):
                dst = v_t[t][:].rearrange("p (h e) -> p h e", h=8, e=65)[:, :, 64:65]
                nc.sync.dma_start(dst, ones_d[:].unsqueeze(2))
            wp_t = [wp_pool.tile([128, D], BF16, name=f"wp{k}", tag=f"wp{k}") for k in range(4)]
            for k in range(4):
                nc.sync.dma_start(wp_t[k][:], wp_d[k * 128:(k + 1) * 128, :])

            def qkv_m(half, x_half, ps_qk, m, wide):
                """q/k features m for one T-half."""
                if wide:
                    ps = ps_qk.tile([128, 1024], F32, name="psqk", tag="psqk")
                    for k in range(8):
                        for n in range(2):
                            nc.tensor.matmul(
                                ps[:, n * 512:(n + 1) * 512],
                                (wqk_t[k][:, m * 128:(m + 1) * 128]),
                                (x_half[k][:, n * 512:(n + 1) * 512]),
                                start=(k == 0), stop=(k == 7),
                                skip_group_check=True,
                            )
                    nc.vector.tensor_copy(qkh[half][m][:], ps[:])
                else:
                    for n in range(2):
                        ps = ps_qk.tile([128, 512], F32, name="psqk", tag="psqk")
                        for k in range(8):
                            nc.tensor.matmul(
                                ps[:],
                                (wqk_t[k][:, m * 128:(m + 1) * 128]),
                                (x_half[k][:, n * 512:(n + 1) * 512]),
                                start=(k == 0), stop=(k == 7),
                                skip_group_check=True,
                            )
                        dst = qkh[half][m][:, n * 512:(n + 1) * 512]
                        nc.vector.tensor_copy(dst, ps[:])

            def v_tt(half, x_half, ps_v, tt, tag):
                psv = ps_v.tile([128, 512], F32, name="psv", tag=tag)
                for k in range(8):
                    nc.tensor.matmul(
                        psv[:],
                        (x_half[k][:, tt * 128:(tt + 1) * 128]),
                        (wv_t[k][:]),
                        start=(k == 0), stop=(k == 7),
                        skip_group_check=True,
                    )
                vt = v_t[half * 8 + tt]
                src = psv[:].rearrange("p (h e) -> p h e", h=8, e=64)
                dst = vt[:].rearrange("p (h e) -> p h e", h=8, e=65)[:, :, 0:64]
                nc.vector.tensor_copy(dst, src)

            def qkv_half(half, x_half, ps_qk, ps_v, n_grp):
                """Project one T-half: q/k into qkh[half], v into v_t."""
                for m in range(8):
                    qkv_m(half, x_half, ps_qk, m, wide=(n_grp == 2))
                for tt in range(8):
                    v_tt(half, x_half, ps_v, tt,
                         "psqk" if ps_v is ps_qk else "psv")

            # ================= phase 1 first half =======================
            with ExitStack() as p1:
                ps_qk = p1.enter_context(tc.tile_pool(name="ps_qk", bufs=2, space="PSUM"))
                ps_v = p1.enter_context(tc.tile_pool(name="ps_v", bufs=2, space="PSUM"))
                # HAM warmup: ~4us of matmuls on the mask tile while the
                # x/wqk DMAs stream in, so real matmuls start at 2.4 GHz
                psw = ps_qk.tile([128, 512], F32, name="psw", tag="psqk")
                for i in range(18):
                    nc.tensor.matmul(
                        psw[:], (mask_t[:, 0:128]), (mask_t[:, 512:1024]),
                        start=(i == 0), stop=(i == 17), skip_group_check=True,
                    )
                qkv_half(0, x_h0, ps_qk, ps_v, n_grp=2)

            # x second half: emit DMAs now so they run during attention
            x_h1 = [x_pool.tile([128, 1024], BF16, name=f"x{k}", tag=f"x{k}") for k in range(8)]
            for k in range(8):
                nc.sync.dma_start(x_h1[k][:], xT_d[k * 128:(k + 1) * 128, 1024:2048])

            # ====== attention + second-half projection + output proj ====
            with ExitStack() as p2:
                ps_s = p2.enter_context(tc.tile_pool(name="ps_s", bufs=2, space="PSUM"))
                ps_av = p2.enter_context(tc.tile_pool(name="ps_av", bufs=3, space="PSUM"))
                pt_pool = p2.enter_context(tc.tile_pool(name="pt", bufs=3))
                r_pool = p2.enter_context(tc.tile_pool(name="recip", bufs=2))

                def attn_chunk(c, pair):
                    """Both heads of `pair` for query chunk c."""
                    npieces = 4 * c + 4
                    hA, hB = 2 * pair, 2 * pair + 1
                    avA = ps_av.tile([65, 512], F32, name="avA", tag="av")
                    avB = ps_av.tile([65, 512], F32, name="avB", tag="av")
                    qh, qo = c // 2, (c % 2) * 512
                    for j in range(npieces):
                        kh, ko = j // 8, (j % 8) * 128
                        # diagonal blocks: columns left of the diagonal are
                        # all-masked -> only compute q >= off
                        off = (j - 4 * c) * 128 if j // 4 == c else 0
                        w = 512 - off
                        s = ps_s.tile([128, 1024], F32, name="s", tag="s")
                        nc.tensor.matmul(
                            s[:, off:512],
                            (qkh[kh][4 + pair][0:64, ko:ko + 128]),
                            (qkh[qh][pair][0:64, qo + off:qo + 512]),
                            start=True, stop=True, skip_group_check=True,
                        )
                        nc.tensor.matmul(
                            s[:, 512 + off:1024],
                            (qkh[kh][4 + pair][64:128, ko:ko + 128]),
                            (qkh[qh][pair][64:128, qo + off:qo + 512]),
                            start=True, stop=True, skip_group_check=True,
                        )
                        pt = pt_pool.tile([128, 1024], BF16, name="pt", tag="pt")
                        if off == 0:
                            nc.scalar.activation(pt[:], s[:], AF.Exp, scale=0.125)
                        else:
                            sv = s[:].rearrange("p (t q) -> p t q", t=2, q=512)
                            pv = pt[:].rearrange("p (t q) -> p t q", t=2, q=512)
                            nc.scalar.activation(
                                pv[:, :, off:512], sv[:, :, off:512],
                                AF.Exp, scale=0.125,
                            )
                        if j // 4 == c:  # diagonal block -> causal mask
                            p = j - 4 * c
                            for half in range(2):
                                nc.vector.tensor_tensor(
                                    pt[:, half * 512 + off:half * 512 + 512],
                                    pt[:, half * 512 + off:half * 512 + 512],
                                    mask_t[:, p * 512 + off:(p + 1) * 512],
                                    op=OP.mult,
                                )
                        nc.tensor.matmul(
                            avA[:, off:512],
                            (v_t[j][:, hA * 65:(hA + 1) * 65]),
                            (pt[:, off:512]),
                            start=(j == 0), stop=(j == npieces - 1),
                            skip_group_check=True,
                        )
                        nc.tensor.matmul(
                            avB[:, off:512],
                            (v_t[j][:, hB * 65:(hB + 1) * 65]),
                            (pt[:, 512 + off:1024]),
                            start=(j == 0), stop=(j == npieces - 1),
                            skip_group_check=True,
                        )
                    # normalize + evacuate both heads
                    den = r_pool.tile([1, 1024], F32, name="den", tag="den")
                    nc.vector.tensor_copy(den[0:1, 0:512], avA[64:65, :])
                    nc.vector.tensor_copy(den[0:1, 512:1024], avB[64:65, :])
                    rec = r_pool.tile([1, 1024], F32, name="rec", tag="rec")
                    nc.vector.reciprocal_approx_fast(rec[:], den[:])
                    rbA = r_pool.tile([64, 512], F32, name="rbA", tag="rbA")
                    rbB = r_pool.tile([64, 512], F32, name="rbB", tag="rbB")
                    nc.gpsimd.partition_broadcast(rbA[:], rec[0:1, 0:512])
                    nc.gpsimd.partition_broadcast(rbB[:], rec[0:1, 512:1024])
                    nc.vector.tensor_tensor(
                        oc_t[pair][c][0:64, :], avA[0:64, :], rbA[:], op=OP.mult,
                    )
                    nc.vector.tensor_tensor(
                        oc_t[pair][c][64:128, :], avB[0:64, :], rbB[:], op=OP.mult,
                    )

                def proj_col(c, ps_y):
                    """Output projection for query column c."""
                    for m in range(8):
                        psy = ps_y.tile([128, 512], F32, name="psy", tag="psy")
                        for kk in range(4):
                            nc.tensor.matmul(
                                psy[:],
                                (wp_t[kk][:, m * 128:(m + 1) * 128]),
                                (oc_t[kk][c][:]),
                                start=(kk == 0), stop=(kk == 3),
                                skip_group_check=True,
                            )
                        yt = y_pool.tile([128, 512], BF16, name="yst", tag="yst")
                        nc.vector.tensor_copy(yt[:], psy[:])
                        nc.sync.dma_start(
                            yT_d[m * 128:(m + 1) * 128, c * 512:(c + 1) * 512], yt[:]
                        )

                for pair in range(4):
                    attn_chunk(0, pair)
                for pair in range(4):
                    attn_chunk(1, pair)

                # second-half projection: fills PE gaps under the chunk-0/1
                # exps and unblocks chunk 2+
                with ExitStack() as p1b:
                    ps_qk1 = p1b.enter_context(tc.tile_pool(name="ps_qk1", bufs=1, space="PSUM"))
                    for m in range(8):
                        qkv_m(1, x_h1, ps_qk1, m, wide=False)
                    for tt in range(8):
                        v_tt(1, x_h1, ps_qk1, tt, "psqk")

                with ExitStack() as p3:
                    ps_y = p3.enter_context(tc.tile_pool(name="ps_y", bufs=1, space="PSUM"))
                    proj_col(0, ps_y)
                    for pair in range(4):
                        attn_chunk(2, pair)
                    proj_col(1, ps_y)
                    for pair in range(2):
                        attn_chunk(3, pair)
                    proj_col(2, ps_y)
                    for pair in range(2, 4):
                        attn_chunk(3, pair)

            # final column after the attention scope closes: PSUM is free,
            # so give the accumulators room to pipeline
            with ExitStack() as p4:
                ps_y2 = p4.enter_context(tc.tile_pool(name="ps_y2", bufs=3, space="PSUM"))
                for m in range(8):
                    psy = ps_y2.tile([128, 512], F32, name="psy", tag="psy")
                    for kk in range(4):
                        nc.tensor.matmul(
                            psy[:],
                            (wp_t[kk][:, m * 128:(m + 1) * 128]),
                            (oc_t[kk][3][:]),
                            start=(kk == 0), stop=(kk == 3),
                            skip_group_check=True,
                        )
                    yt = y_pool.tile([128, 512], BF16, name="yst", tag="yst")
                    nc.vector.tensor_copy(yt[:], psy[:])
                    nc.sync.dma_start(
                        yT_d[m * 128:(m + 1) * 128, 3 * 512:4 * 512], yt[:]
                    )

    nc.compile()
    return nc


def _make_mask():
    mask = np.zeros((128, 2048), dtype=np.float32)
    kk = np.arange(128)[:, None]
    q = np.arange(512)[None, :]
    for p in range(4):
        d = 128 * p
        mask[:, p * 512:(p + 1) * 512] = ((q - d) >= kk).astype(np.float32)
    return mask


def kernel(x, w_qkv, w_proj):
    global LAST_RESULTS, _CACHED
    x = np.asarray(x, dtype=np.float32)
    w_qkv = np.asarray(w_qkv, dtype=np.float32)
    w_proj = np.asarray(w_proj, dtype=np.float32)
    B = x.shape[0]

    if _CACHED is None:
        _CACHED = build_program()
    nc = _CACHED

    bf16 = ml_dtypes.bfloat16
    mask = _make_mask()
    in_maps = []
    for c in range(8):
        b, g = c // 2, c % 2
        wq = w_qkv[g * GF:(g + 1) * GF, :]                # [512, 1024]
        wk = w_qkv[D + g * GF: D + (g + 1) * GF, :]
        wv = w_qkv[2 * D + g * GF: 2 * D + (g + 1) * GF, :]
        in_maps.append({
            "xT": np.ascontiguousarray(x[b].T).astype(bf16),
            "wqkT": np.ascontiguousarray(np.concatenate([wq, wk], axis=0).T).astype(bf16),
            "wvT": np.ascontiguousarray(wv.T).astype(bf16),
            "wpT": np.ascontiguousarray(w_proj[:, g * GF:(g + 1) * GF].T).astype(bf16),
            "mask": mask.astype(bf16),
            "ones8": np.ones((128, 8), bf16),
        })

    res = run_bass_kernel_spmd(nc, in_maps, core_ids=list(range(8)))
    LAST_RESULTS = res

    y = np.empty_like(x)
    for b in range(B):
        yT = (res.results[2 * b]["yT"].astype(np.float32)
              + res.results[2 * b + 1]["yT"].astype(np.float32))
        y[b] = yT.T
    return y


# revision 14
# speedup vs baseline: 1.0380x; 1.0332x over previous
"""Causal self-attention on 8 TRN2 NeuronCores.

Sharding: core c -> (batch b = c//2, head-group g = c%2).
B=4, T=2048, D=1024, 16 heads x 64. Each core computes attention for its
batch and its 8 heads, plus the partial output projection for those heads;
the host sums the two partial projections per batch.

Device layouts (host pre-transposes everything, all bf16):
  xT    [1024, 2048]  x[b].T
  wqkT  [1024, 1024]  cols 0..511 q-feats, 512..1023 k-feats (group g)
  wvT   [1024, 512]   v-feats (group g)
  wpT   [512, 1024]   w_proj[:, g*512:(g+1)*512].T
  mask  [128, 2048]   4 causal patterns of [128,512] for diag offsets 0/128/256/384
Output: yT [1024, 2048] bf16 partial y[b].T (sum over this core's heads).

Attention runs in S^T orientation (k on partitions, q on free dim). The two
heads of a pair (2p, 2p+1) live on SBUF partitions 0-63 / 64-127 of the
same qk tiles, and each k-block piece computes BOTH heads' scores into one
[128,1024] PSUM tile (A in cols 0-511, B in 512-1023) so a single ACT exp
covers the pair, the s tile double-buffers in 4 PSUM banks, and the K=64
matmuls land on PE row-tiles (0,0)/(64,0). P^T V accumulates per head with
an all-ones V column producing softmax denominators in row 64.
Normalization: DVE fast-reciprocal on both heads' denominators at once ->
GpSimd partition-broadcast -> DVE multiply on evacuation.

Scheduling: query chunks 0/1 only need the first half of q/k/v, so the
second half of the QKV projection is emitted after chunk 1 and fills PE
gaps while ACT grinds exps; the output projection for a query column is
emitted right after its chunk completes for the same reason.
"""

import sys

for _p in ("/opt/pypackages", "/opt/trn_rl_repo"):
    if _p not in sys.path:
        sys.path.insert(0, _p)

from contextlib import ExitStack

import ml_dtypes
import numpy as np

import concourse.bass as bass
import concourse.tile as tile
from concourse import bacc, mybir
from concourse.bass_utils import run_bass_kernel_spmd

F32 = mybir.dt.float32
BF16 = mybir.dt.bfloat16
AF = mybir.ActivationFunctionType
OP = mybir.AluOpType

D = 1024
T = 2048
NH_LOC = 8          # heads per core
DH = 64
GF = NH_LOC * DH    # 512 features per group

LAST_RESULTS = None
_CACHED = None


def build_program():
    nc = bacc.Bacc("TRN2", target_bir_lowering=False, debug=False)

    xT_d = nc.dram_tensor("xT", [D, T], BF16, kind="ExternalInput").ap()
    wqk_d = nc.dram_tensor("wqkT", [D, 2 * GF], BF16, kind="ExternalInput").ap()
    wv_d = nc.dram_tensor("wvT", [D, GF], BF16, kind="ExternalInput").ap()
    wp_d = nc.dram_tensor("wpT", [GF, D], BF16, kind="ExternalInput").ap()
    mask_d = nc.dram_tensor("mask", [128, 2048], BF16, kind="ExternalInput").ap()
    ones_d = nc.dram_tensor("ones8", [128, 8], BF16, kind="ExternalInput").ap()
    yT_d = nc.dram_tensor("yT", [D, T], BF16, kind="ExternalOutput").ap()

    with tile.TileContext(nc) as tc:
        with ExitStack() as octx:
            # ---- persistent pools --------------------------------------
            qk_pool = octx.enter_context(tc.tile_pool(name="qkT", bufs=1))
            v_pool = octx.enter_context(tc.tile_pool(name="vN", bufs=1))
            o_pool = octx.enter_context(tc.tile_pool(name="outT", bufs=1))
            c_pool = octx.enter_context(tc.tile_pool(name="const", bufs=1))
            w_pool = octx.enter_context(tc.tile_pool(name="wqk", bufs=1))
            wv_pool = octx.enter_context(tc.tile_pool(name="wv", bufs=1))
            wp_pool = octx.enter_context(tc.tile_pool(name="wp", bufs=1))
            x_pool = octx.enter_context(tc.tile_pool(name="xT", bufs=2))
            y_pool = octx.enter_context(tc.tile_pool(name="y", bufs=4))

            # qk: per-half tiles [128,1024]; m 0..3 q-feats, m 4..7 k-feats
            qkh = [[qk_pool.tile([128, 1024], BF16, name=f"qk{h}_{m}", tag=f"qk{h}_{m}")
                    for m in range(8)] for h in range(2)]
            # vN: 16 tiles [128, 520]; cols h*65+0..63 v-feats, col h*65+64 ones
            v_t = [v_pool.tile([128, 8 * (DH + 1)], BF16, name=f"v{t}", tag=f"v{t}") for t in range(16)]
            # attention outputs per (pair, chunk): [128, 512]
            oc_t = [[o_pool.tile([128, 512], BF16, name=f"o{p}_{c}", tag=f"o{p}_{c}")
                     for c in range(4)] for p in range(4)]

            # ---- input DMAs (mask first for PE warmup, then x/wqk
            # interleaved so the first accumulation can start ASAP) -------
            mask_t = c_pool.tile([128, 2048], BF16, name="mask", tag="mask")
            nc.sync.dma_start(mask_t[:], mask_d[:])
            x_h0 = [x_pool.tile([128, 1024], BF16, name=f"x{k}", tag=f"x{k}") for k in range(8)]
            wqk_t = [w_pool.tile([128, 2 * GF], BF16, name=f"wqk{k}", tag=f"wqk{k}") for k in range(8)]
            for k in range(8):
                nc.sync.dma_start(x_h0[k][:], xT_d[k * 128:(k + 1) * 128, 0:1024])
                nc.sync.dma_start(wqk_t[k][:], wqk_d[k * 128:(k + 1) * 128, :])
            wv_t = [wv_pool.tile([128, GF], BF16, name=f"wv{k}", tag=f"wv{k}") for k in range(8)]
            for k in range(8):
                nc.sync.dma_start(wv_t[k][:], wv_d[k * 128:(k + 1) * 128, :])
            for t in range(16):
                dst = v_t[t][:].rearrange("p (h e) -> p h e", h=8, e=65)[:, :, 64:65]
                nc.sync.dma_start(dst, ones_d[:].unsqueeze(2))
            wp_t = [wp_pool.tile([128, D], BF16, name=f"wp{k}", tag=f"wp{k}") for k in range(4)]
            for k in range(4):
                nc.sync.dma_start(wp_t[k][:], wp_d[k * 128:(k + 1) * 128, :])

            def qkv_m(half, x_half, ps_qk, m, wide, ns=(0, 1)):
                """q/k features m for one T-half."""
                if wide:
                    ps = ps_qk.tile([128, 1024], F32, name="psqk", tag="psqk")
                    for k in range(8):
                        for n in range(2):
                            nc.tensor.matmul(
                                ps[:, n * 512:(n + 1) * 512],
                                (wqk_t[k][:, m * 128:(m + 1) * 128]),
                                (x_half[k][:, n * 512:(n + 1) * 512]),
                                start=(k == 0), stop=(k == 7),
                                skip_group_check=True,
                            )
                    nc.vector.tensor_copy(qkh[half][m][:], ps[:])
                else:
                    for n in ns:
                        ps = ps_qk.tile([128, 512], F32, name="psqk", tag="psqk")
                        for k in range(8):
                            nc.tensor.matmul(
                                ps[:],
                                (wqk_t[k][:, m * 128:(m + 1) * 128]),
                                (x_half[k][:, n * 512:(n + 1) * 512]),
                                start=(k == 0), stop=(k == 7),
                                skip_group_check=True,
                            )
                        dst = qkh[half][m][:, n * 512:(n + 1) * 512]
                        nc.vector.tensor_copy(dst, ps[:])

            def v_tt(half, x_half, ps_v, tt, tag):
                psv = ps_v.tile([128, 512], F32, name="psv", tag=tag)
                for k in range(8):
                    nc.tensor.matmul(
                        psv[:],
                        (x_half[k][:, tt * 128:(tt + 1) * 128]),
                        (wv_t[k][:]),
                        start=(k == 0), stop=(k == 7),
                        skip_group_check=True,
                    )
                vt = v_t[half * 8 + tt]
                src = psv[:].rearrange("p (h e) -> p h e", h=8, e=64)
                dst = vt[:].rearrange("p (h e) -> p h e", h=8, e=65)[:, :, 0:64]
                nc.vector.tensor_copy(dst, src)

            def qkv_half(half, x_half, ps_qk, ps_v, n_grp):
                """Project one T-half: q/k into qkh[half], v into v_t."""
                for m in range(8):
                    qkv_m(half, x_half, ps_qk, m, wide=(n_grp == 2))
                for tt in range(8):
                    v_tt(half, x_half, ps_v, tt,
                         "psqk" if ps_v is ps_qk else "psv")

            # ================= phase 1 first half =======================
            with ExitStack() as p1:
                ps_qk = p1.enter_context(tc.tile_pool(name="ps_qk", bufs=2, space="PSUM"))
                ps_v = p1.enter_context(tc.tile_pool(name="ps_v", bufs=2, space="PSUM"))
                # HAM warmup: ~4us of matmuls on the mask tile while the
                # x/wqk DMAs stream in, so real matmuls start at 2.4 GHz
                psw = ps_qk.tile([128, 512], F32, name="psw", tag="psqk")
                for i in range(18):
                    nc.tensor.matmul(
                        psw[:], (mask_t[:, 0:128]), (mask_t[:, 512:1024]),
                        start=(i == 0), stop=(i == 17), skip_group_check=True,
                    )
                qkv_half(0, x_h0, ps_qk, ps_v, n_grp=2)

            # x second half: emit DMAs now so they run during attention
            x_h1 = [x_pool.tile([128, 1024], BF16, name=f"x{k}", tag=f"x{k}") for k in range(8)]
            for k in range(8):
                nc.sync.dma_start(x_h1[k][:], xT_d[k * 128:(k + 1) * 128, 1024:2048])

            # ====== attention + second-half projection + output proj ====
            with ExitStack() as p2:
                ps_s = p2.enter_context(tc.tile_pool(name="ps_s", bufs=2, space="PSUM"))
                ps_av = p2.enter_context(tc.tile_pool(name="ps_av", bufs=3, space="PSUM"))
                pt_pool = p2.enter_context(tc.tile_pool(name="pt", bufs=3))
                r_pool = p2.enter_context(tc.tile_pool(name="recip", bufs=2))

                def attn_chunk(c, pair):
                    """Both heads of `pair` for query chunk c."""
                    npieces = 4 * c + 4
                    hA, hB = 2 * pair, 2 * pair + 1
                    avA = ps_av.tile([65, 512], F32, name="avA", tag="av")
                    avB = ps_av.tile([65, 512], F32, name="avB", tag="av")
                    qh, qo = c // 2, (c % 2) * 512
                    for j in range(npieces):
                        kh, ko = j // 8, (j % 8) * 128
                        # diagonal blocks: columns left of the diagonal are
                        # all-masked -> only compute q >= off
                        off = (j - 4 * c) * 128 if j // 4 == c else 0
                        w = 512 - off
                        s = ps_s.tile([128, 1024], F32, name="s", tag="s")
                        nc.tensor.matmul(
                            s[:, off:512],
                            (qkh[kh][4 + pair][0:64, ko:ko + 128]),
                            (qkh[qh][pair][0:64, qo + off:qo + 512]),
                            start=True, stop=True, skip_group_check=True,
                        )
                        nc.tensor.matmul(
                            s[:, 512 + off:1024],
                            (qkh[kh][4 + pair][64:128, ko:ko + 128]),
                            (qkh[qh][pair][64:128, qo + off:qo + 512]),
                            start=True, stop=True, skip_group_check=True,
                        )
                        pt = pt_pool.tile([128, 1024], BF16, name="pt", tag="pt")
                        if off == 0:
                            nc.scalar.activation(pt[:], s[:], AF.Exp, scale=0.125)
                        else:
                            sv = s[:].rearrange("p (t q) -> p t q", t=2, q=512)
                            pv = pt[:].rearrange("p (t q) -> p t q", t=2, q=512)
                            nc.scalar.activation(
                                pv[:, :, off:512], sv[:, :, off:512],
                                AF.Exp, scale=0.125,
                            )
                        if j // 4 == c:  # diagonal block -> causal mask
                            p = j - 4 * c
                            for half in range(2):
                                nc.vector.tensor_tensor(
                                    pt[:, half * 512 + off:half * 512 + 512],
                                    pt[:, half * 512 + off:half * 512 + 512],
                                    mask_t[:, p * 512 + off:(p + 1) * 512],
                                    op=OP.mult,
                                )
                        nc.tensor.matmul(
                            avA[:, off:512],
                            (v_t[j][:, hA * 65:(hA + 1) * 65]),
                            (pt[:, off:512]),
                            start=(j == 0), stop=(j == npieces - 1),
                            skip_group_check=True,
                        )
                        nc.tensor.matmul(
                            avB[:, off:512],
                            (v_t[j][:, hB * 65:(hB + 1) * 65]),
                            (pt[:, 512 + off:1024]),
                            start=(j == 0), stop=(j == npieces - 1),
                            skip_group_check=True,
                        )
                    # normalize + evacuate both heads
                    den = r_pool.tile([1, 1024], F32, name="den", tag="den")
                    nc.vector.tensor_copy(den[0:1, 0:512], avA[64:65, :])
                    nc.vector.tensor_copy(den[0:1, 512:1024], avB[64:65, :])
                    rec = r_pool.tile([1, 1024], F32, name="rec", tag="rec")
                    nc.vector.reciprocal_approx_fast(rec[:], den[:])
                    rbA = r_pool.tile([64, 512], F32, name="rbA", tag="rbA")
                    rbB = r_pool.tile([64, 512], F32, name="rbB", tag="rbB")
                    nc.gpsimd.partition_broadcast(rbA[:], rec[0:1, 0:512])
                    nc.gpsimd.partition_broadcast(rbB[:], rec[0:1, 512:1024])
                    nc.vector.tensor_tensor(
                        oc_t[pair][c][0:64, :], avA[0:64, :], rbA[:], op=OP.mult,
                    )
                    nc.vector.tensor_tensor(
                        oc_t[pair][c][64:128, :], avB[0:64, :], rbB[:], op=OP.mult,
                    )

                def proj_col(c, ps_y):
                    """Output projection for query column c."""
                    for m in range(8):
                        psy = ps_y.tile([128, 512], F32, name="psy", tag="psy")
                        for kk in range(4):
                            nc.tensor.matmul(
                                psy[:],
                                (wp_t[kk][:, m * 128:(m + 1) * 128]),
                                (oc_t[kk][c][:]),
                                start=(kk == 0), stop=(kk == 3),
                                skip_group_check=True,
                            )
                        yt = y_pool.tile([128, 512], BF16, name="yst", tag="yst")
                        nc.vector.tensor_copy(yt[:], psy[:])
                        nc.sync.dma_start(
                            yT_d[m * 128:(m + 1) * 128, c * 512:(c + 1) * 512], yt[:]
                        )

                for pair in range(4):
                    attn_chunk(0, pair)
                for pair in range(4):
                    attn_chunk(1, pair)

                # second-half projection: fills PE gaps under the chunk-0/1
                # exps and unblocks chunk 2+. The parts only chunk 3 needs
                # (k blocks 12-15, v tiles 12-15) are emitted after chunk-2
                # attention so they fill its ACT-bound window instead.
                with ExitStack() as p1b:
                    ps_qk1 = p1b.enter_context(tc.tile_pool(name="ps_qk1", bufs=1, space="PSUM"))
                    for m in range(8):
                        qkv_m(1, x_h1, ps_qk1, m, wide=False, ns=(0,))
                    for tt in range(4):
                        v_tt(1, x_h1, ps_qk1, tt, "psqk")
                    for pair in range(4):
                        attn_chunk(2, pair)
                    for m in range(8):
                        qkv_m(1, x_h1, ps_qk1, m, wide=False, ns=(1,))
                    for tt in range(4, 8):
                        v_tt(1, x_h1, ps_qk1, tt, "psqk")

                with ExitStack() as p3:
                    ps_y = p3.enter_context(tc.tile_pool(name="ps_y", bufs=1, space="PSUM"))
                    proj_col(0, ps_y)
                    proj_col(1, ps_y)
                    for pair in range(2):
                        attn_chunk(3, pair)
                    proj_col(2, ps_y)
                    for pair in range(2, 4):
                        attn_chunk(3, pair)

            # final column after the attention scope closes: PSUM is free,
            # so give the accumulators room to pipeline
            with ExitStack() as p4:
                ps_y2 = p4.enter_context(tc.tile_pool(name="ps_y2", bufs=3, space="PSUM"))
                for m in range(8):
                    psy = ps_y2.tile([128, 512], F32, name="psy", tag="psy")
                    for kk in range(4):
                        nc.tensor.matmul(
                            psy[:],
                            (wp_t[kk][:, m * 128:(m + 1) * 128]),
                            (oc_t[kk][3][:]),
                            start=(kk == 0), stop=(kk == 3),
                            skip_group_check=True,
                        )
                    yt = y_pool.tile([128, 512], BF16, name="yst", tag="yst")
                    nc.vector.tensor_copy(yt[:], psy[:])
                    nc.sync.dma_start(
                        yT_d[m * 128:(m + 1) * 128, 3 * 512:4 * 512], yt[:]
                    )

    nc.compile()
    return nc


def _make_mask():
    mask = np.zeros((128, 2048), dtype=np.float32)
    kk = np.arange(128)[:, None]
    q = np.arange(512)[None, :]
    for p in range(4):
        d = 128 * p
        mask[:, p * 512:(p + 1) * 512] = ((q - d) >= kk).astype(np.float32)
    return mask


def kernel(x, w_qkv, w_proj):
    global LAST_RESULTS, _CACHED
    x = np.asarray(x, dtype=np.float32)
    w_qkv = np.asarray(w_qkv, dtype=np.float32)
    w_proj = np.asarray(w_proj, dtype=np.float32)
    B = x.shape[0]

    if _CACHED is None:
        _CACHED = build_program()
    nc = _CACHED

    bf16 = ml_dtypes.bfloat16
    mask = _make_mask()
    in_maps = []
    for c in range(8):
        b, g = c // 2, c % 2
        wq = w_qkv[g * GF:(g + 1) * GF, :]                # [512, 1024]
        wk = w_qkv[D + g * GF: D + (g + 1) * GF, :]
        wv = w_qkv[2 * D + g * GF: 2 * D + (g + 1) * GF, :]
        in_maps.append({
            "xT": np.ascontiguousarray(x[b].T).astype(bf16),
            "wqkT": np.ascontiguousarray(np.concatenate([wq, wk], axis=0).T).astype(bf16),
            "wvT": np.ascontiguousarray(wv.T).astype(bf16),
            "wpT": np.ascontiguousarray(w_proj[:, g * GF:(g + 1) * GF].T).astype(bf16),
            "mask": mask.astype(bf16),
            "ones8": np.ones((128, 8), bf16),
        })

    res = run_bass_kernel_spmd(nc, in_maps, core_ids=list(range(8)))
    LAST_RESULTS = res

    y = np.empty_like(x)
    for b in range(B):
        yT = (res.results[2 * b]["yT"].astype(np.float32)
              + res.results[2 * b + 1]["yT"].astype(np.float32))
        y[b] = yT.T
    return y


# revision 15
# speedup vs baseline: 1.0533x; 1.0148x over previous
"""Causal self-attention on 8 TRN2 NeuronCores.

Sharding: core c -> (batch b = c//2, head-group g = c%2).
B=4, T=2048, D=1024, 16 heads x 64. Each core computes attention for its
batch and its 8 heads, plus the partial output projection for those heads;
the host sums the two partial projections per batch.

Device layouts (host pre-transposes everything, all bf16):
  xT    [1024, 2048]  x[b].T
  wqkT  [1024, 1024]  cols 0..511 q-feats, 512..1023 k-feats (group g)
  wvT   [1024, 512]   v-feats (group g)
  wpT   [512, 1024]   w_proj[:, g*512:(g+1)*512].T
  mask  [128, 2048]   4 causal patterns of [128,512] for diag offsets 0/128/256/384
Output: yT [1024, 2048] bf16 partial y[b].T (sum over this core's heads).

Attention runs in S^T orientation (k on partitions, q on free dim). The two
heads of a pair (2p, 2p+1) live on SBUF partitions 0-63 / 64-127 of the
same qk tiles, and each k-block piece computes BOTH heads' scores into one
[128,1024] PSUM tile (A in cols 0-511, B in 512-1023) so a single ACT exp
covers the pair, the s tile double-buffers in 4 PSUM banks, and the K=64
matmuls land on PE row-tiles (0,0)/(64,0). P^T V accumulates per head with
an all-ones V column producing softmax denominators in row 64.
Normalization: DVE fast-reciprocal on both heads' denominators at once ->
GpSimd partition-broadcast -> DVE multiply on evacuation.

Scheduling: query chunks 0/1 only need the first half of q/k/v, so the
second half of the QKV projection is emitted after chunk 1 and fills PE
gaps while ACT grinds exps; the output projection for a query column is
emitted right after its chunk completes for the same reason.
"""

import sys

for _p in ("/opt/pypackages", "/opt/trn_rl_repo"):
    if _p not in sys.path:
        sys.path.insert(0, _p)

from contextlib import ExitStack

import ml_dtypes
import numpy as np

import concourse.bass as bass
import concourse.tile as tile
from concourse import bacc, mybir
from concourse.bass_utils import run_bass_kernel_spmd

F32 = mybir.dt.float32
BF16 = mybir.dt.bfloat16
AF = mybir.ActivationFunctionType
OP = mybir.AluOpType

D = 1024
T = 2048
NH_LOC = 8          # heads per core
DH = 64
GF = NH_LOC * DH    # 512 features per group

LAST_RESULTS = None
_CACHED = None


def build_program():
    nc = bacc.Bacc("TRN2", target_bir_lowering=False, debug=False)

    xT_d = nc.dram_tensor("xT", [D, T], BF16, kind="ExternalInput").ap()
    wqk_d = nc.dram_tensor("wqkT", [D, 2 * GF], BF16, kind="ExternalInput").ap()
    wv_d = nc.dram_tensor("wvT", [D, GF], BF16, kind="ExternalInput").ap()
    wp_d = nc.dram_tensor("wpT", [GF, D], BF16, kind="ExternalInput").ap()
    mask_d = nc.dram_tensor("mask", [128, 2048], BF16, kind="ExternalInput").ap()
    ones_d = nc.dram_tensor("ones8", [128, 8], BF16, kind="ExternalInput").ap()
    yT_d = nc.dram_tensor("yT", [D, T], BF16, kind="ExternalOutput").ap()

    with tile.TileContext(nc) as tc:
        with ExitStack() as octx:
            # ---- persistent pools --------------------------------------
            qk_pool = octx.enter_context(tc.tile_pool(name="qkT", bufs=1))
            v_pool = octx.enter_context(tc.tile_pool(name="vN", bufs=1))
            o_pool = octx.enter_context(tc.tile_pool(name="outT", bufs=1))
            c_pool = octx.enter_context(tc.tile_pool(name="const", bufs=1))
            w_pool = octx.enter_context(tc.tile_pool(name="wqk", bufs=1))
            wv_pool = octx.enter_context(tc.tile_pool(name="wv", bufs=1))
            wp_pool = octx.enter_context(tc.tile_pool(name="wp", bufs=1))
            x_pool = octx.enter_context(tc.tile_pool(name="xT", bufs=2))
            y_pool = octx.enter_context(tc.tile_pool(name="y", bufs=4))

            # qk: per-half tiles [128,1024]; m 0..3 q-feats, m 4..7 k-feats
            qkh = [[qk_pool.tile([128, 1024], BF16, name=f"qk{h}_{m}", tag=f"qk{h}_{m}")
                    for m in range(8)] for h in range(2)]
            # vN: 16 tiles [128, 520]; cols h*65+0..63 v-feats, col h*65+64 ones
            v_t = [v_pool.tile([128, 8 * (DH + 1)], BF16, name=f"v{t}", tag=f"v{t}") for t in range(16)]
            # attention outputs per (pair, chunk): [128, 512]
            oc_t = [[o_pool.tile([128, 512], BF16, name=f"o{p}_{c}", tag=f"o{p}_{c}")
                     for c in range(4)] for p in range(4)]

            # ---- input DMAs (mask first for PE warmup, then x/wqk
            # interleaved so the first accumulation can start ASAP) -------
            mask_t = c_pool.tile([128, 2048], BF16, name="mask", tag="mask")
            nc.sync.dma_start(mask_t[:], mask_d[:])
            x_h0 = [x_pool.tile([128, 1024], BF16, name=f"x{k}", tag=f"x{k}") for k in range(8)]
            wqk_t = [w_pool.tile([128, 2 * GF], BF16, name=f"wqk{k}", tag=f"wqk{k}") for k in range(8)]
            for k in range(8):
                nc.sync.dma_start(x_h0[k][:], xT_d[k * 128:(k + 1) * 128, 0:1024])
                nc.sync.dma_start(wqk_t[k][:], wqk_d[k * 128:(k + 1) * 128, :])
            wv_t = [wv_pool.tile([128, GF], BF16, name=f"wv{k}", tag=f"wv{k}") for k in range(8)]
            for k in range(8):
                nc.sync.dma_start(wv_t[k][:], wv_d[k * 128:(k + 1) * 128, :])
            for t in range(16):
                dst = v_t[t][:].rearrange("p (h e) -> p h e", h=8, e=65)[:, :, 64:65]
                nc.sync.dma_start(dst, ones_d[:].unsqueeze(2))
            wp_t = [wp_pool.tile([128, D], BF16, name=f"wp{k}", tag=f"wp{k}") for k in range(4)]
            for k in range(4):
                nc.sync.dma_start(wp_t[k][:], wp_d[k * 128:(k + 1) * 128, :])

            def qkv_m(half, x_half, ps_qk, m, wide, ns=(0, 1)):
                """q/k features m for one T-half."""
                if wide:
                    ps = ps_qk.tile([128, 1024], F32, name="psqk", tag="psqk")
                    for k in range(8):
                        for n in range(2):
                            nc.tensor.matmul(
                                ps[:, n * 512:(n + 1) * 512],
                                (wqk_t[k][:, m * 128:(m + 1) * 128]),
                                (x_half[k][:, n * 512:(n + 1) * 512]),
                                start=(k == 0), stop=(k == 7),
                                skip_group_check=True,
                            )
                    nc.vector.tensor_copy(qkh[half][m][:], ps[:])
                else:
                    for n in ns:
                        ps = ps_qk.tile([128, 512], F32, name="psqk", tag="psqk")
                        for k in range(8):
                            nc.tensor.matmul(
                                ps[:],
                                (wqk_t[k][:, m * 128:(m + 1) * 128]),
                                (x_half[k][:, n * 512:(n + 1) * 512]),
                                start=(k == 0), stop=(k == 7),
                                skip_group_check=True,
                            )
                        dst = qkh[half][m][:, n * 512:(n + 1) * 512]
                        nc.vector.tensor_copy(dst, ps[:])

            def v_tt(half, x_half, ps_v, tt, tag):
                psv = ps_v.tile([128, 512], F32, name="psv", tag=tag)
                for k in range(8):
                    nc.tensor.matmul(
                        psv[:],
                        (x_half[k][:, tt * 128:(tt + 1) * 128]),
                        (wv_t[k][:]),
                        start=(k == 0), stop=(k == 7),
                        skip_group_check=True,
                    )
                vt = v_t[half * 8 + tt]
                src = psv[:].rearrange("p (h e) -> p h e", h=8, e=64)
                dst = vt[:].rearrange("p (h e) -> p h e", h=8, e=65)[:, :, 0:64]
                nc.vector.tensor_copy(dst, src)

            def qkv_half(half, x_half, ps_qk, ps_v, n_grp):
                """Project one T-half: q/k into qkh[half], v into v_t."""
                for m in range(8):
                    qkv_m(half, x_half, ps_qk, m, wide=(n_grp == 2))
                for tt in range(8):
                    v_tt(half, x_half, ps_v, tt,
                         "psqk" if ps_v is ps_qk else "psv")

            # ================= phase 1 first half =======================
            with ExitStack() as p1:
                ps_qk = p1.enter_context(tc.tile_pool(name="ps_qk", bufs=2, space="PSUM"))
                ps_v = p1.enter_context(tc.tile_pool(name="ps_v", bufs=2, space="PSUM"))
                # HAM warmup: ~4us of matmuls on the mask tile while the
                # x/wqk DMAs stream in, so real matmuls start at 2.4 GHz
                psw = ps_qk.tile([128, 512], F32, name="psw", tag="psqk")
                for i in range(18):
                    nc.tensor.matmul(
                        psw[:], (mask_t[:, 0:128]), (mask_t[:, 512:1024]),
                        start=(i == 0), stop=(i == 17), skip_group_check=True,
                    )
                qkv_half(0, x_h0, ps_qk, ps_v, n_grp=2)

            # x second half: emit DMAs now so they run during attention
            x_h1 = [x_pool.tile([128, 1024], BF16, name=f"x{k}", tag=f"x{k}") for k in range(8)]
            for k in range(8):
                nc.sync.dma_start(x_h1[k][:], xT_d[k * 128:(k + 1) * 128, 1024:2048])

            # ====== attention + second-half projection + output proj ====
            with ExitStack() as p2:
                ps_s = p2.enter_context(tc.tile_pool(name="ps_s", bufs=2, space="PSUM"))
                ps_av = p2.enter_context(tc.tile_pool(name="ps_av", bufs=3, space="PSUM"))
                pt_pool = p2.enter_context(tc.tile_pool(name="pt", bufs=3))
                r_pool = p2.enter_context(tc.tile_pool(name="recip", bufs=2))

                def attn_chunk(c, pair):
                    """Both heads of `pair` for query chunk c."""
                    npieces = 4 * c + 4
                    hA, hB = 2 * pair, 2 * pair + 1
                    avA = ps_av.tile([65, 512], F32, name="avA", tag="av")
                    avB = ps_av.tile([65, 512], F32, name="avB", tag="av")
                    qh, qo = c // 2, (c % 2) * 512
                    for j in range(npieces):
                        kh, ko = j // 8, (j % 8) * 128
                        # diagonal blocks: columns left of the diagonal are
                        # all-masked -> only compute q >= off
                        off = (j - 4 * c) * 128 if j // 4 == c else 0
                        w = 512 - off
                        s = ps_s.tile([128, 1024], F32, name="s", tag="s")
                        nc.tensor.matmul(
                            s[:, off:512],
                            (qkh[kh][4 + pair][0:64, ko:ko + 128]),
                            (qkh[qh][pair][0:64, qo + off:qo + 512]),
                            start=True, stop=True, skip_group_check=True,
                        )
                        nc.tensor.matmul(
                            s[:, 512 + off:1024],
                            (qkh[kh][4 + pair][64:128, ko:ko + 128]),
                            (qkh[qh][pair][64:128, qo + off:qo + 512]),
                            start=True, stop=True, skip_group_check=True,
                        )
                        pt = pt_pool.tile([128, 1024], BF16, name="pt", tag="pt")
                        if off == 0:
                            nc.scalar.activation(pt[:], s[:], AF.Exp, scale=0.125)
                        else:
                            sv = s[:].rearrange("p (t q) -> p t q", t=2, q=512)
                            pv = pt[:].rearrange("p (t q) -> p t q", t=2, q=512)
                            nc.scalar.activation(
                                pv[:, :, off:512], sv[:, :, off:512],
                                AF.Exp, scale=0.125,
                            )
                        if j // 4 == c:  # diagonal block -> causal mask
                            p = j - 4 * c
                            for half in range(2):
                                nc.vector.tensor_tensor(
                                    pt[:, half * 512 + off:half * 512 + 512],
                                    pt[:, half * 512 + off:half * 512 + 512],
                                    mask_t[:, p * 512 + off:(p + 1) * 512],
                                    op=OP.mult,
                                )
                        nc.tensor.matmul(
                            avA[:, off:512],
                            (v_t[j][:, hA * 65:(hA + 1) * 65]),
                            (pt[:, off:512]),
                            start=(j == 0), stop=(j == npieces - 1),
                            skip_group_check=True,
                        )
                        nc.tensor.matmul(
                            avB[:, off:512],
                            (v_t[j][:, hB * 65:(hB + 1) * 65]),
                            (pt[:, 512 + off:1024]),
                            start=(j == 0), stop=(j == npieces - 1),
                            skip_group_check=True,
                        )
                    # normalize + evacuate; per-head chains so head A's
                    # broadcast/multiply overlaps head B's reciprocal
                    den = r_pool.tile([1, 1024], F32, name="den", tag="den")
                    rec = r_pool.tile([1, 1024], F32, name="rec", tag="rec")
                    rbA = r_pool.tile([64, 512], F32, name="rbA", tag="rbA")
                    rbB = r_pool.tile([64, 512], F32, name="rbB", tag="rbB")
                    nc.vector.tensor_copy(den[0:1, 0:512], avA[64:65, :])
                    nc.vector.reciprocal_approx_fast(rec[0:1, 0:512], den[0:1, 0:512])
                    nc.gpsimd.partition_broadcast(rbA[:], rec[0:1, 0:512])
                    nc.vector.tensor_copy(den[0:1, 512:1024], avB[64:65, :])
                    nc.vector.reciprocal_approx_fast(rec[0:1, 512:1024], den[0:1, 512:1024])
                    nc.vector.tensor_tensor(
                        oc_t[pair][c][0:64, :], avA[0:64, :], rbA[:], op=OP.mult,
                    )
                    nc.gpsimd.partition_broadcast(rbB[:], rec[0:1, 512:1024])
                    nc.vector.tensor_tensor(
                        oc_t[pair][c][64:128, :], avB[0:64, :], rbB[:], op=OP.mult,
                    )

                def proj_col(c, ps_y):
                    """Output projection for query column c."""
                    for m in range(8):
                        psy = ps_y.tile([128, 512], F32, name="psy", tag="psy")
                        for kk in range(4):
                            nc.tensor.matmul(
                                psy[:],
                                (wp_t[kk][:, m * 128:(m + 1) * 128]),
                                (oc_t[kk][c][:]),
                                start=(kk == 0), stop=(kk == 3),
                                skip_group_check=True,
                            )
                        yt = y_pool.tile([128, 512], BF16, name="yst", tag="yst")
                        nc.vector.tensor_copy(yt[:], psy[:])
                        nc.sync.dma_start(
                            yT_d[m * 128:(m + 1) * 128, c * 512:(c + 1) * 512], yt[:]
                        )

                for pair in range(4):
                    attn_chunk(0, pair)
                for pair in range(4):
                    attn_chunk(1, pair)

                # second-half projection: fills PE gaps under the chunk-0/1
                # exps and unblocks chunk 2+. The parts only chunk 3 needs
                # (k blocks 12-15, v tiles 12-15) are emitted after chunk-2
                # attention so they fill its ACT-bound window instead.
                with ExitStack() as p1b:
                    ps_qk1 = p1b.enter_context(tc.tile_pool(name="ps_qk1", bufs=1, space="PSUM"))
                    for m in range(8):
                        qkv_m(1, x_h1, ps_qk1, m, wide=False, ns=(0,))
                    for tt in range(4):
                        v_tt(1, x_h1, ps_qk1, tt, "psqk")
                    for pair in range(4):
                        attn_chunk(2, pair)
                    for m in range(8):
                        qkv_m(1, x_h1, ps_qk1, m, wide=False, ns=(1,))
                    for tt in range(4, 8):
                        v_tt(1, x_h1, ps_qk1, tt, "psqk")

                with ExitStack() as p3:
                    ps_y = p3.enter_context(tc.tile_pool(name="ps_y", bufs=1, space="PSUM"))
                    proj_col(0, ps_y)
                    proj_col(1, ps_y)
                    for pair in range(2):
                        attn_chunk(3, pair)
                    proj_col(2, ps_y)
                    for pair in range(2, 4):
                        attn_chunk(3, pair)

            # final column after the attention scope closes: PSUM is free,
            # so give the accumulators room to pipeline
            with ExitStack() as p4:
                ps_y2 = p4.enter_context(tc.tile_pool(name="ps_y2", bufs=3, space="PSUM"))
                for m in range(8):
                    psy = ps_y2.tile([128, 512], F32, name="psy", tag="psy")
                    for kk in range(4):
                        nc.tensor.matmul(
                            psy[:],
                            (wp_t[kk][:, m * 128:(m + 1) * 128]),
                            (oc_t[kk][3][:]),
                            start=(kk == 0), stop=(kk == 3),
                            skip_group_check=True,
                        )
                    yt = y_pool.tile([128, 512], BF16, name="yst", tag="yst")
                    nc.vector.tensor_copy(yt[:], psy[:])
                    nc.sync.dma_start(
                        yT_d[m * 128:(m + 1) * 128, 3 * 512:4 * 512], yt[:]
                    )

    nc.compile()
    return nc


def _make_mask():
    mask = np.zeros((128, 2048), dtype=np.float32)
    kk = np.arange(128)[:, None]
    q = np.arange(512)[None, :]
    for p in range(4):
        d = 128 * p
        mask[:, p * 512:(p + 1) * 512] = ((q - d) >= kk).astype(np.float32)
    return mask


def kernel(x, w_qkv, w_proj):
    global LAST_RESULTS, _CACHED
    x = np.asarray(x, dtype=np.float32)
    w_qkv = np.asarray(w_qkv, dtype=np.float32)
    w_proj = np.asarray(w_proj, dtype=np.float32)
    B = x.shape[0]

    if _CACHED is None:
        _CACHED = build_program()
    nc = _CACHED

    bf16 = ml_dtypes.bfloat16
    mask = _make_mask()
    in_maps = []
    for c in range(8):
        b, g = c // 2, c % 2
        wq = w_qkv[g * GF:(g + 1) * GF, :]                # [512, 1024]
        wk = w_qkv[D + g * GF: D + (g + 1) * GF, :]
        wv = w_qkv[2 * D + g * GF: 2 * D + (g + 1) * GF, :]
        in_maps.append({
            "xT": np.ascontiguousarray(x[b].T).astype(bf16),
            "wqkT": np.ascontiguousarray(np.concatenate([wq, wk], axis=0).T).astype(bf16),
            "wvT": np.ascontiguousarray(wv.T).astype(bf16),
            "wpT": np.ascontiguousarray(w_proj[:, g * GF:(g + 1) * GF].T).astype(bf16),
            "mask": mask.astype(bf16),
            "ones8": np.ones((128, 8), bf16),
        })

    res = run_bass_kernel_spmd(nc, in_maps, core_ids=list(range(8)))
    LAST_RESULTS = res

    y = np.empty_like(x)
    for b in range(B):
        yT = (res.results[2 * b]["yT"].astype(np.float32)
              + res.results[2 * b + 1]["yT"].astype(np.float32))
        y[b] = yT.T
    return y
